# revision 17
# baseline (speedup 1.0000x reference)
"""GQA attention prefill (L=2048, D=4096, 32 Q heads / 8 KV heads, RoPE, causal)
on 8 Trainium2 NeuronCores, tensor-parallel over kv-head groups.

Per core c: q heads 4c..4c+3, kv head c.
  - QKV projections: lhsT = x^T blocks, rhs = concat(wq*scale | wk | wv) (f32r)
  - RoPE in natural [L, hd] layout with even/odd-separated head dims
    (host permutes weight columns so pairs land in free-axis halves)
  - scores S[q, keys] = QT^T @ KT (f32r), causal-tiled; exp on ScalarE with
    row-sum accumulation; P normalized (fp16) and DMA-transposed per block
  - attnout^T = sum_m V_m^T @ P^T_m (fp16); out partial = attnT^T @ wo (f32r)
  - host: sum the 8 partial outputs; assemble k/v returns (GQA repeat)
"""

import numpy as np

L = 2048
DIM = 4096
H = 32
KV = 8
HD = 128
NCORES = 8
NH = H // NCORES  # 4
SCALE = HD ** -0.5
ROPE_BASE = 10000.0
NLB = L // 128  # 16
NDC = DIM // 128  # 32
QW = NH * HD  # 512
CW = QW + 2 * HD  # 768 = q|k|v concat width

PERM = np.concatenate([np.arange(0, HD, 2), np.arange(1, HD, 2)])
INVPERM = np.empty(HD, dtype=np.int64)
INVPERM[PERM] = np.arange(HD)

# packed P^T strip offsets: strip m holds q-cols m*128 .. 2048
PT_OFF = [0] * (NLB + 1)
for _m in range(NLB):
    PT_OFF[_m + 1] = PT_OFF[_m] + (NLB - _m) * 128
PT_W = PT_OFF[NLB]  # 17408

_CACHE = {}


def _build():
    if "nc" in _CACHE:
        return _CACHE["nc"]
    import concourse.mybir as mybir
    from concourse import bacc
    from concourse.tile import TileContext

    F32 = mybir.dt.float32
    F32R = mybir.dt.float32r
    FP16 = mybir.dt.float16
    EXP = mybir.ActivationFunctionType.Exp

    nc = bacc.Bacc("TRN2", target_bir_lowering=False, debug=False, num_devices=NCORES)

    xT_d = nc.declare_dram_parameter("xT", [DIM, L], FP16, isOutput=False)
    wqkv_d = nc.declare_dram_parameter("wqkv", [DIM, CW], FP16, isOutput=False)
    wo_d = nc.declare_dram_parameter("wo", [QW, DIM], F32R, isOutput=False)
    cos_d = nc.declare_dram_parameter("cosT", [L, 64], F32, isOutput=False)
    sin_d = nc.declare_dram_parameter("sinT", [L, 64], F32, isOutput=False)
    tri_d = nc.declare_dram_parameter("tri", [128, 128], F32, isOutput=False)
    triT_d = nc.declare_dram_parameter("triT", [128, 128], F32, isOutput=False)
    id_d = nc.declare_dram_parameter("ident", [128, 128], F32, isOutput=False)
    out_d = nc.declare_dram_parameter("out_p", [L, DIM], F32, isOutput=True)
    k_d = nc.declare_dram_parameter("k_out", [L, HD], F32, isOutput=True)
    v_d = nc.declare_dram_parameter("v_out", [L, HD], F32, isOutput=True)

    with TileContext(nc) as tc:
        with (
            tc.tile_pool(name="outer", bufs=1) as po,      # attnT, tri, ident
            tc.tile_pool(name="qkv", bufs=1) as pq,        # QT, KT, V
        ):
            tri_t = po.tile([128, 128], F32)
            bias_t = po.tile([128, 1], F32, name="bias_t")
            id16_t = po.tile([128, 128], FP16, name="id16_t")
            nc.vector.memset(bias_t[:, :], -5.0)
            id_t = po.tile([128, 128], F32)
            nc.sync.dma_start(out=tri_t[:, :], in_=tri_d[:, :])
            triT_t = po.tile([128, 128], F32, name="triT_t")
            nc.sync.dma_start(out=triT_t[:, :], in_=triT_d[:, :])
            ones16 = po.tile([128, 1], FP16, name="ones16")
            nc.vector.memset(ones16[:, :], 1.0)
            nc.sync.dma_start(out=id_t[:, :], in_=id_d[:, :])
            nc.vector.tensor_copy(out=id16_t[:, :], in_=id_t[:, :])
            QT = [pq.tile([128, L], F32R, tag=f"QT{h}", name=f"QT{h}") for h in range(NH)]
            KT = pq.tile([128, L], F32R)
            V = pq.tile([128, NLB, 128], FP16)

            # ---------------- phase 1: projections + rope ----------------
            with (
                tc.tile_pool(name="ph1", bufs=1) as p1,
                tc.tile_pool(name="ph1s", bufs=2) as p1s,
                tc.tile_pool(name="xs", bufs=2) as pxs,
                tc.tile_pool(name="pjp", bufs=2, space="PSUM") as pjp,
                tc.tile_pool(name="tpp", bufs=2, space="PSUM") as ptp,
            ):
                wq_sb = [p1.tile([128, 2, CW], FP16, tag=f"wq{i}", name=f"wq{i}") for i in range(16)]
                for i in range(16):
                    nc.sync.dma_start(
                        out=wq_sb[i][:, :, :],
                        in_=wqkv_d[i * 256:(i + 1) * 256, :].rearrange(
                            "(k p) c -> p k c", p=128
                        ),
                    )
                cos_t = p1.tile([128, NLB, 64], F32)
                sin_t = p1.tile([128, NLB, 64], F32)
                nc.sync.dma_start(
                    out=cos_t[:, :, :],
                    in_=cos_d[:, :].rearrange("(b p) c -> p b c", p=128),
                )
                nc.sync.dma_start(
                    out=sin_t[:, :, :],
                    in_=sin_d[:, :].rearrange("(b p) c -> p b c", p=128),
                )

                for lb in range(NLB):
                    xts = []
                    for half in range(2):
                        xt = pxs.tile([128, 16, 128], FP16, tag="xT")
                        nc.sync.dma_start(
                            out=xt[:, :, :],
                            in_=xT_d[half * 2048:(half + 1) * 2048,
                                     lb * 128:(lb + 1) * 128].rearrange(
                                "(k p) l -> p k l", p=128
                            ),
                        )
                        xts.append(xt)
                    pj = pjp.tile([128, CW], F32)
                    for dc in range(NDC):
                        lhsT = xts[dc // 16][:, dc % 16, :]
                        w = wq_sb[dc // 2]
                        nc.tensor.matmul(
                            pj[:, 0:512], lhsT, w[:, dc % 2, 0:512],
                            start=(dc == 0), stop=(dc == NDC - 1),
                        )
                        nc.tensor.matmul(
                            pj[:, 512:768], lhsT, w[:, dc % 2, 512:768],
                            start=(dc == 0), stop=(dc == NDC - 1),
                        )

                    # --- rope on q (4 heads batched) and k, from psum ---
                    cb = cos_t[:, lb, :].rearrange("p (u c) -> p u c", u=1)
                    sb = sin_t[:, lb, :].rearrange("p (u c) -> p u c", u=1)
                    cb4 = cb.broadcast_to([128, NH, 64])
                    sb4 = sb.broadcast_to([128, NH, 64])
                    q4 = pj[:, 0:512].rearrange("p (h t c) -> p h t c", h=NH, c=64)
                    qe, qo = q4[:, :, 0, :], q4[:, :, 1, :]
                    tA = p1s.tile([128, NH, 64], F32, tag="tA")
                    tB = p1s.tile([128, NH, 64], F32, tag="tB")
                    qro = p1s.tile([128, 512], F32, tag="qro")
                    qro4 = qro[:, :].rearrange("p (h t c) -> p h t c", h=NH, c=64)
                    nc.vector.tensor_mul(tA[:, :, :], qe, cb4)
                    nc.vector.tensor_mul(tB[:, :, :], qo, sb4)
                    nc.vector.tensor_sub(qro4[:, :, 0, :], tA[:, :, :], tB[:, :, :])
                    nc.vector.tensor_mul(tA[:, :, :], qe, sb4)
                    nc.vector.tensor_mul(tB[:, :, :], qo, cb4)
                    nc.vector.tensor_add(qro4[:, :, 1, :], tA[:, :, :], tB[:, :, :])

                    k2 = pj[:, 512:640].rearrange("p (t c) -> p t c", c=64)
                    ke, ko = k2[:, 0, :], k2[:, 1, :]
                    kA = p1s.tile([128, 64], F32, tag="kA")
                    kB = p1s.tile([128, 64], F32, tag="kB")
                    kro = p1s.tile([128, 128], F32, tag="kro")
                    nc.vector.tensor_mul(kA[:, :], ke, cb[:, 0, :])
                    nc.vector.tensor_mul(kB[:, :], ko, sb[:, 0, :])
                    nc.vector.tensor_sub(kro[:, 0:64], kA[:, :], kB[:, :])
                    nc.vector.tensor_mul(kA[:, :], ke, sb[:, 0, :])
                    nc.vector.tensor_mul(kB[:, :], ko, cb[:, 0, :])
                    nc.vector.tensor_add(kro[:, 64:128], kA[:, :], kB[:, :])
                    nc.sync.dma_start(
                        out=k_d[lb * 128:(lb + 1) * 128, :], in_=kro[:, :]
                    )

                    vst = p1s.tile([128, 128], F32, tag="vst")
                    nc.scalar.copy(out=vst[:, :], in_=pj[:, 640:768])
                    nc.sync.dma_start(
                        out=v_d[lb * 128:(lb + 1) * 128, :], in_=vst[:, :]
                    )
                    nc.scalar.copy(out=V[:, lb, :], in_=pj[:, 640:768])

                    # transposes: q heads + k into QT/KT (f32 via PE)
                    for h in range(NH):
                        tp = ptp.tile([128, 128], F32)
                        nc.tensor.transpose(
                            tp[:, :], qro[:, h * 128:(h + 1) * 128], id_t[:, :]
                        )
                        nc.vector.tensor_copy(
                            out=QT[h][:, lb * 128:(lb + 1) * 128], in_=tp[:, :]
                        )
                    tp = ptp.tile([128, 128], F32)
                    nc.tensor.transpose(tp[:, :], kro[:, :], id_t[:, :])
                    nc.vector.tensor_copy(
                        out=KT[:, lb * 128:(lb + 1) * 128], in_=tp[:, :]
                    )

            # ---------------- phase 2: attention ----------------
            with tc.tile_pool(name="wo", bufs=1) as pwo:
                attnT = [pwo.tile([128, L], F32R, tag=f"attnT{h}", name=f"attnT{h}") for h in range(NH)]
                wo_sb = [pwo.tile([128, DIM], F32R, tag=f"wo{h}", name=f"wo{h}") for h in range(NH)]
                for h in range(NH):
                    nc.sync.dma_start(
                        out=wo_sb[h][:, :], in_=wo_d[h * 128:(h + 1) * 128, :]
                    )

                with (
                    tc.tile_pool(name="pt", bufs=1) as ppt,
                    tc.tile_pool(name="nrm", bufs=2) as pnrm,
                    tc.tile_pool(name="stp", bufs=2, space="PSUM") as pstp,
                    tc.tile_pool(name="dnp", bufs=2, space="PSUM") as pdnp,
                    tc.tile_pool(name="atp", bufs=2, space="PSUM") as patp,
                ):
                    PT = ppt.tile([128, PT_W], FP16)
                    for h in range(NH):
                        # ST pass: PT strips = exp(K_m^T @ Q - 5), fp16, packed
                        for m in range(NLB):
                            wt = L - m * 128
                            for c0 in range(0, wt, 1024):
                                cw = min(1024, wt - c0)
                                ST = pstp.tile([128, 1024], F32, name="ST", tag="ST")
                                for d0 in range(0, cw, 512):
                                    dw = min(512, cw - d0)
                                    nc.tensor.matmul(
                                        ST[:, d0:d0 + dw],
                                        KT[:, m * 128:(m + 1) * 128],
                                        QT[h][:, m * 128 + c0 + d0:
                                               m * 128 + c0 + d0 + dw],
                                        start=True, stop=True,
                                    )
                                if c0 == 0:  # diagonal block: keys>q masked
                                    nc.vector.tensor_add(
                                        ST[:, 0:128], ST[:, 0:128], triT_t[:, :]
                                    )
                                nc.scalar.activation(
                                    PT[:, PT_OFF[m] + c0:PT_OFF[m] + c0 + cw],
                                    ST[:, 0:cw], EXP, bias=bias_t[:, :],
                                )
                        # denominator + attn accumulation + normalize, per 512-col chunk
                        for cc in range(4):
                            mmax = min(4 * cc + 3, NLB - 1)
                            den = pdnp.tile([1, 512], F32, name="den", tag="den")
                            at = patp.tile([128, 512], F32, name="at", tag="at")
                            for m in range(mmax + 1):
                                qs = max(cc * 512, m * 128)
                                qe_ = cc * 512 + 512
                                pts = PT[:, PT_OFF[m] + qs - m * 128:
                                         PT_OFF[m] + qe_ - m * 128]
                                nc.tensor.matmul(
                                    den[0:1, qs - cc * 512:512], ones16[:, :], pts,
                                    start=(m == 0), stop=(m == mmax),
                                )
                                nc.tensor.matmul(
                                    at[:, qs - cc * 512:512], V[:, m, :], pts,
                                    start=(m == 0), stop=(m == mmax),
                                )
                            cden = pnrm.tile([1, 512], F32, name="cden", tag="cden")
                            nc.vector.reciprocal_approx_fast(cden[:, :], den[0:1, :])
                            rbc = pnrm.tile([128, 512], F32, name="rbc", tag="rbc")
                            nc.gpsimd.partition_broadcast(rbc[:, :], cden[0:1, :])
                            nc.vector.tensor_mul(
                                attnT[h][:, cc * 512:(cc + 1) * 512],
                                at[:, :], rbc[:, :],
                            )

            # ---------------- phase 3: out partial = attnT^T @ wo ----------------
                with (
                    tc.tile_pool(name="os", bufs=2) as pos,
                    tc.tile_pool(name="osp", bufs=2, space="PSUM") as posp,
                ):
                    for lb in range(NLB):
                        for half in range(2):
                            op = posp.tile([128, 2048], F32)
                            for h in range(NH):
                                for ck in range(4):
                                    n0 = half * 2048 + ck * 512
                                    nc.tensor.matmul(
                                        op[:, ck * 512:(ck + 1) * 512],
                                        attnT[h][:, lb * 128:(lb + 1) * 128],
                                        wo_sb[h][:, n0:n0 + 512],
                                        start=(h == 0), stop=(h == NH - 1),
                                    )
                            ost = pos.tile([128, 2048], F32, tag="ost")
                            if half == 0:
                                nc.vector.tensor_copy(out=ost[:, :], in_=op[:, :])
                            else:
                                nc.scalar.copy(out=ost[:, :], in_=op[:, :])
                            nc.sync.dma_start(
                                out=out_d[lb * 128:(lb + 1) * 128,
                                          half * 2048:(half + 1) * 2048],
                                in_=ost[:, :],
                            )

    nc.compile()
    _CACHE["nc"] = nc
    return nc


def _host_prep(x, wq, wk, wv, wo):
    x = np.asarray(x, dtype=np.float32)
    wq = np.asarray(wq, dtype=np.float32)
    wk = np.asarray(wk, dtype=np.float32)
    wv = np.asarray(wv, dtype=np.float32)
    wo = np.asarray(wo, dtype=np.float32)

    xT = np.ascontiguousarray(x[0].T)  # [DIM, L]
    xT16 = xT.astype(np.float16)

    pos = np.arange(L, dtype=np.float32)
    inv_freq = (ROPE_BASE ** (-np.arange(0, HD, 2, dtype=np.float32) / HD)).astype(
        np.float32
    )
    ang = pos[:, None] * inv_freq[None, :]
    cosT = np.cos(ang).astype(np.float32)
    sinT = np.sin(ang).astype(np.float32)

    ii = np.arange(128)
    tri = np.where(ii[None, :] <= ii[:, None], 0.0, -1e9).astype(np.float32)
    triT = np.ascontiguousarray(tri.T)
    ident = np.eye(128, dtype=np.float32)

    in_maps = []
    for c in range(NCORES):
        qcols = np.concatenate(
            [c * QW + h * HD + PERM for h in range(NH)]
        )
        wq_c = (wq[:, qcols] * np.float32(SCALE)).astype(np.float32)
        wk_c = wk[:, c * HD + PERM]
        wv_c = wv[:, c * HD:(c + 1) * HD]
        wqkv_c = np.ascontiguousarray(
            np.concatenate([wq_c, wk_c, wv_c], axis=1)
        ).astype(np.float16)
        wo_c = np.ascontiguousarray(wo[c * QW:(c + 1) * QW, :])
        in_maps.append(
            {
                "xT": xT16,
                "wqkv": wqkv_c,
                "wo": wo_c,
                "cosT": cosT,
                "sinT": sinT,
                "tri": tri,
                "triT": triT,
                "ident": ident,
            }
        )
    return in_maps


def _assemble(results):
    out = np.zeros((L, DIM), dtype=np.float64)
    for r in results:
        out += r["out_p"].astype(np.float64)
    out = out.astype(np.float32).reshape(1, L, DIM)

    k8 = np.stack([r["k_out"][:, INVPERM] for r in results])  # [8, L, HD]
    v8 = np.stack([r["v_out"] for r in results])
    k = np.repeat(k8, NH, axis=0).reshape(1, H, L, HD).astype(np.float32)
    v = np.repeat(v8, NH, axis=0).reshape(1, H, L, HD).astype(np.float32)
    return out, k, v


def _run(inputs, trace=False):
    from concourse.bass_utils import run_bass_kernel_spmd

    nc = _build()
    in_maps = _host_prep(
        inputs["x"], inputs["wq"], inputs["wk"], inputs["wv"], inputs["wo"]
    )
    res = run_bass_kernel_spmd(
        nc, in_maps, core_ids=list(range(NCORES)), trace=trace
    )
    return _assemble(res.results), res


def kernel(x, wq, wk, wv, wo, mask):
    (out, k, v), _ = _run(
        {"x": x, "wq": wq, "wk": wk, "wv": wv, "wo": wo}
    )
    return out, k, v


# revision 18
# speedup vs baseline: 1.1599x; 1.1599x over previous
"""GQA attention prefill (L=2048, D=4096, 32 Q heads / 8 KV heads, RoPE, causal)
on 8 Trainium2 NeuronCores, tensor-parallel over kv-head groups.

Per core c: q heads 4c..4c+3, kv head c.
  - QKV projections: lhsT = x^T blocks, rhs = concat(wq*scale | wk | wv) (f32r)
  - RoPE in natural [L, hd] layout with even/odd-separated head dims
    (host permutes weight columns so pairs land in free-axis halves)
  - scores S[q, keys] = QT^T @ KT (f32r), causal-tiled; exp on ScalarE with
    row-sum accumulation; P normalized (fp16) and DMA-transposed per block
  - attnout^T = sum_m V_m^T @ P^T_m (fp16); out partial = attnT^T @ wo (f32r)
  - host: sum the 8 partial outputs; assemble k/v returns (GQA repeat)
"""

import numpy as np

L = 2048
DIM = 4096
H = 32
KV = 8
HD = 128
NCORES = 8
NH = H // NCORES  # 4
SCALE = HD ** -0.5
ROPE_BASE = 10000.0
NLB = L // 128  # 16
NDC = DIM // 128  # 32
QW = NH * HD  # 512
CW = QW + 2 * HD  # 768 = q|k|v concat width

PERM = np.concatenate([np.arange(0, HD, 2), np.arange(1, HD, 2)])
INVPERM = np.empty(HD, dtype=np.int64)
INVPERM[PERM] = np.arange(HD)

# packed P^T strip offsets: strip m holds q-cols m*128 .. 2048
PT_OFF = [0] * (NLB + 1)
for _m in range(NLB):
    PT_OFF[_m + 1] = PT_OFF[_m] + (NLB - _m) * 128
PT_W = PT_OFF[NLB]  # 17408

_CACHE = {}


def _build():
    if "nc" in _CACHE:
        return _CACHE["nc"]
    import concourse.mybir as mybir
    from concourse import bacc
    from concourse.tile import TileContext

    F32 = mybir.dt.float32
    F32R = mybir.dt.float32r
    FP16 = mybir.dt.float16
    EXP = mybir.ActivationFunctionType.Exp

    nc = bacc.Bacc("TRN2", target_bir_lowering=False, debug=False, num_devices=NCORES)

    xT_d = nc.declare_dram_parameter("xT", [DIM, L], FP16, isOutput=False)
    wqkv_d = nc.declare_dram_parameter("wqkv", [DIM, CW], FP16, isOutput=False)
    wo_d = nc.declare_dram_parameter("wo", [QW, DIM], F32R, isOutput=False)
    cos_d = nc.declare_dram_parameter("cosT", [L, 64], F32, isOutput=False)
    sin_d = nc.declare_dram_parameter("sinT", [L, 64], F32, isOutput=False)
    tri_d = nc.declare_dram_parameter("tri", [128, 128], F32, isOutput=False)
    triT_d = nc.declare_dram_parameter("triT", [128, 128], F32, isOutput=False)
    id_d = nc.declare_dram_parameter("ident", [128, 128], F32, isOutput=False)
    out_d = nc.declare_dram_parameter("out_p", [L, DIM], F32, isOutput=True)
    k_d = nc.declare_dram_parameter("k_out", [L, HD], F32, isOutput=True)
    v_d = nc.declare_dram_parameter("v_out", [L, HD], F32, isOutput=True)

    with TileContext(nc) as tc:
        with (
            tc.tile_pool(name="outer", bufs=1) as po,      # attnT, tri, ident
            tc.tile_pool(name="qkv", bufs=1) as pq,        # QT, KT, V
        ):
            tri_t = po.tile([128, 128], F32)
            bias_t = po.tile([128, 1], F32, name="bias_t")
            id16_t = po.tile([128, 128], FP16, name="id16_t")
            nc.vector.memset(bias_t[:, :], -5.0)
            id_t = po.tile([128, 128], F32)
            nc.sync.dma_start(out=tri_t[:, :], in_=tri_d[:, :])
            triT_t = po.tile([128, 128], F32, name="triT_t")
            nc.sync.dma_start(out=triT_t[:, :], in_=triT_d[:, :])
            ones16 = po.tile([128, 1], FP16, name="ones16")
            nc.vector.memset(ones16[:, :], 1.0)
            nc.sync.dma_start(out=id_t[:, :], in_=id_d[:, :])
            nc.vector.tensor_copy(out=id16_t[:, :], in_=id_t[:, :])
            QT = [pq.tile([128, L], F32R, tag=f"QT{h}", name=f"QT{h}") for h in range(NH)]
            KT = pq.tile([128, L], F32R)
            V = pq.tile([128, NLB, 128], FP16)

            # ---------------- phase 1: projections + rope ----------------
            with (
                tc.tile_pool(name="ph1", bufs=1) as p1,
                tc.tile_pool(name="ph1s", bufs=2) as p1s,
                tc.tile_pool(name="xs", bufs=2) as pxs,
                tc.tile_pool(name="pjp", bufs=2, space="PSUM") as pjp,
                tc.tile_pool(name="tpp", bufs=2, space="PSUM") as ptp,
            ):
                wq_sb = [p1.tile([128, 2, CW], FP16, tag=f"wq{i}", name=f"wq{i}") for i in range(16)]
                for i in range(16):
                    nc.sync.dma_start(
                        out=wq_sb[i][:, :, :],
                        in_=wqkv_d[i * 256:(i + 1) * 256, :].rearrange(
                            "(k p) c -> p k c", p=128
                        ),
                    )
                cos_t = p1.tile([128, NLB, 64], F32)
                sin_t = p1.tile([128, NLB, 64], F32)
                nc.sync.dma_start(
                    out=cos_t[:, :, :],
                    in_=cos_d[:, :].rearrange("(b p) c -> p b c", p=128),
                )
                nc.sync.dma_start(
                    out=sin_t[:, :, :],
                    in_=sin_d[:, :].rearrange("(b p) c -> p b c", p=128),
                )

                for lb in range(NLB):
                    xts = []
                    for half in range(2):
                        xt = pxs.tile([128, 16, 128], FP16, tag="xT")
                        nc.sync.dma_start(
                            out=xt[:, :, :],
                            in_=xT_d[half * 2048:(half + 1) * 2048,
                                     lb * 128:(lb + 1) * 128].rearrange(
                                "(k p) l -> p k l", p=128
                            ),
                        )
                        xts.append(xt)
                    pj = pjp.tile([128, CW], F32)
                    for dc in range(NDC):
                        lhsT = xts[dc // 16][:, dc % 16, :]
                        w = wq_sb[dc // 2]
                        nc.tensor.matmul(
                            pj[:, 0:512], lhsT, w[:, dc % 2, 0:512],
                            start=(dc == 0), stop=(dc == NDC - 1),
                        )
                        nc.tensor.matmul(
                            pj[:, 512:768], lhsT, w[:, dc % 2, 512:768],
                            start=(dc == 0), stop=(dc == NDC - 1),
                        )

                    # --- rope on q (4 heads batched) and k, from psum ---
                    cb = cos_t[:, lb, :].rearrange("p (u c) -> p u c", u=1)
                    sb = sin_t[:, lb, :].rearrange("p (u c) -> p u c", u=1)
                    cb4 = cb.broadcast_to([128, NH, 64])
                    sb4 = sb.broadcast_to([128, NH, 64])
                    q4 = pj[:, 0:512].rearrange("p (h t c) -> p h t c", h=NH, c=64)
                    qe, qo = q4[:, :, 0, :], q4[:, :, 1, :]
                    tA = p1s.tile([128, NH, 64], F32, tag="tA")
                    tB = p1s.tile([128, NH, 64], F32, tag="tB")
                    qro = p1s.tile([128, 512], F32, tag="qro")
                    qro4 = qro[:, :].rearrange("p (h t c) -> p h t c", h=NH, c=64)
                    nc.vector.tensor_mul(tA[:, :, :], qe, cb4)
                    nc.vector.tensor_mul(tB[:, :, :], qo, sb4)
                    nc.vector.tensor_sub(qro4[:, :, 0, :], tA[:, :, :], tB[:, :, :])
                    nc.vector.tensor_mul(tA[:, :, :], qe, sb4)
                    nc.vector.tensor_mul(tB[:, :, :], qo, cb4)
                    nc.vector.tensor_add(qro4[:, :, 1, :], tA[:, :, :], tB[:, :, :])

                    k2 = pj[:, 512:640].rearrange("p (t c) -> p t c", c=64)
                    ke, ko = k2[:, 0, :], k2[:, 1, :]
                    kA = p1s.tile([128, 64], F32, tag="kA")
                    kB = p1s.tile([128, 64], F32, tag="kB")
                    kro = p1s.tile([128, 128], F32, tag="kro")
                    nc.vector.tensor_mul(kA[:, :], ke, cb[:, 0, :])
                    nc.vector.tensor_mul(kB[:, :], ko, sb[:, 0, :])
                    nc.vector.tensor_sub(kro[:, 0:64], kA[:, :], kB[:, :])
                    nc.vector.tensor_mul(kA[:, :], ke, sb[:, 0, :])
                    nc.vector.tensor_mul(kB[:, :], ko, cb[:, 0, :])
                    nc.vector.tensor_add(kro[:, 64:128], kA[:, :], kB[:, :])
                    nc.sync.dma_start(
                        out=k_d[lb * 128:(lb + 1) * 128, :], in_=kro[:, :]
                    )

                    vst = p1s.tile([128, 128], F32, tag="vst")
                    nc.scalar.copy(out=vst[:, :], in_=pj[:, 640:768])
                    nc.sync.dma_start(
                        out=v_d[lb * 128:(lb + 1) * 128, :], in_=vst[:, :]
                    )
                    nc.scalar.copy(out=V[:, lb, :], in_=pj[:, 640:768])

                    # transposes: q heads + k into QT/KT (f32 via PE)
                    for h in range(NH):
                        tp = ptp.tile([128, 128], F32)
                        nc.tensor.transpose(
                            tp[:, :], qro[:, h * 128:(h + 1) * 128], id_t[:, :]
                        )
                        nc.vector.tensor_copy(
                            out=QT[h][:, lb * 128:(lb + 1) * 128], in_=tp[:, :]
                        )
                    tp = ptp.tile([128, 128], F32)
                    nc.tensor.transpose(tp[:, :], kro[:, :], id_t[:, :])
                    nc.vector.tensor_copy(
                        out=KT[:, lb * 128:(lb + 1) * 128], in_=tp[:, :]
                    )

            # ---------------- phase 2: attention ----------------
            with tc.tile_pool(name="wo", bufs=1) as pwo:
                attnT = [pwo.tile([128, L], F32R, tag=f"attnT{h}", name=f"attnT{h}") for h in range(NH)]
                wo_sb = [pwo.tile([128, DIM], F32R, tag=f"wo{h}", name=f"wo{h}") for h in range(NH)]
                for h in range(NH):
                    nc.sync.dma_start(
                        out=wo_sb[h][:, :], in_=wo_d[h * 128:(h + 1) * 128, :]
                    )

                with (
                    tc.tile_pool(name="pt", bufs=1) as ppt,
                    tc.tile_pool(name="nrm", bufs=2) as pnrm,
                    tc.tile_pool(name="stp", bufs=2, space="PSUM") as pstp,
                    tc.tile_pool(name="dnp", bufs=2, space="PSUM") as pdnp,
                    tc.tile_pool(name="atp", bufs=2, space="PSUM") as patp,
                ):
                    PT = ppt.tile([128, PT_W], FP16)
                    for h in range(NH):
                        # ST pass: PT strips = exp(K_m^T @ Q - 5), fp16, packed
                        for m in range(NLB):
                            wt = L - m * 128
                            for c0 in range(0, wt, 512):
                                cw = min(512, wt - c0)
                                ST = pstp.tile([128, 512], F32, name="ST", tag="ST")
                                nc.tensor.matmul(
                                    ST[:, 0:cw],
                                    KT[:, m * 128:(m + 1) * 128],
                                    QT[h][:, m * 128 + c0:m * 128 + c0 + cw],
                                    start=True, stop=True,
                                )
                                if c0 == 0:  # diagonal block: keys>q masked
                                    nc.vector.tensor_add(
                                        ST[:, 0:128], ST[:, 0:128], triT_t[:, :]
                                    )
                                nc.scalar.activation(
                                    PT[:, PT_OFF[m] + c0:PT_OFF[m] + c0 + cw],
                                    ST[:, 0:cw], EXP, bias=bias_t[:, :],
                                )
                        # denominator + attn accumulation + normalize, per 512-col chunk
                        for cc in range(4):
                            mmax = min(4 * cc + 3, NLB - 1)
                            den = pdnp.tile([1, 512], F32, name="den", tag="den")
                            at = patp.tile([128, 512], F32, name="at", tag="at")
                            for m in range(mmax + 1):
                                qs = max(cc * 512, m * 128)
                                qe_ = cc * 512 + 512
                                pts = PT[:, PT_OFF[m] + qs - m * 128:
                                         PT_OFF[m] + qe_ - m * 128]
                                nc.tensor.matmul(
                                    den[0:1, qs - cc * 512:512], ones16[:, :], pts,
                                    start=(m == 0), stop=(m == mmax),
                                )
                                nc.tensor.matmul(
                                    at[:, qs - cc * 512:512], V[:, m, :], pts,
                                    start=(m == 0), stop=(m == mmax),
                                )
                            cden = pnrm.tile([1, 512], F32, name="cden", tag="cden")
                            nc.vector.reciprocal_approx_fast(cden[:, :], den[0:1, :])
                            rbc = pnrm.tile([128, 512], F32, name="rbc", tag="rbc")
                            nc.gpsimd.partition_broadcast(rbc[:, :], cden[0:1, :])
                            nc.vector.tensor_mul(
                                attnT[h][:, cc * 512:(cc + 1) * 512],
                                at[:, :], rbc[:, :],
                            )

            # ---------------- phase 3: out partial = attnT^T @ wo ----------------
                with (
                    tc.tile_pool(name="os", bufs=2) as pos,
                    tc.tile_pool(name="osp", bufs=2, space="PSUM") as posp,
                ):
                    for lb in range(NLB):
                        for half in range(2):
                            op = posp.tile([128, 2048], F32)
                            for h in range(NH):
                                for ck in range(4):
                                    n0 = half * 2048 + ck * 512
                                    nc.tensor.matmul(
                                        op[:, ck * 512:(ck + 1) * 512],
                                        attnT[h][:, lb * 128:(lb + 1) * 128],
                                        wo_sb[h][:, n0:n0 + 512],
                                        start=(h == 0), stop=(h == NH - 1),
                                    )
                            ost = pos.tile([128, 2048], F32, tag="ost")
                            if half == 0:
                                nc.vector.tensor_copy(out=ost[:, :], in_=op[:, :])
                            else:
                                nc.scalar.copy(out=ost[:, :], in_=op[:, :])
                            nc.sync.dma_start(
                                out=out_d[lb * 128:(lb + 1) * 128,
                                          half * 2048:(half + 1) * 2048],
                                in_=ost[:, :],
                            )

    nc.compile()
    _CACHE["nc"] = nc
    return nc


def _host_prep(x, wq, wk, wv, wo):
    x = np.asarray(x, dtype=np.float32)
    wq = np.asarray(wq, dtype=np.float32)
    wk = np.asarray(wk, dtype=np.float32)
    wv = np.asarray(wv, dtype=np.float32)
    wo = np.asarray(wo, dtype=np.float32)

    xT = np.ascontiguousarray(x[0].T)  # [DIM, L]
    xT16 = xT.astype(np.float16)

    pos = np.arange(L, dtype=np.float32)
    inv_freq = (ROPE_BASE ** (-np.arange(0, HD, 2, dtype=np.float32) / HD)).astype(
        np.float32
    )
    ang = pos[:, None] * inv_freq[None, :]
    cosT = np.cos(ang).astype(np.float32)
    sinT = np.sin(ang).astype(np.float32)

    ii = np.arange(128)
    tri = np.where(ii[None, :] <= ii[:, None], 0.0, -1e9).astype(np.float32)
    triT = np.ascontiguousarray(tri.T)
    ident = np.eye(128, dtype=np.float32)

    in_maps = []
    for c in range(NCORES):
        qcols = np.concatenate(
            [c * QW + h * HD + PERM for h in range(NH)]
        )
        wq_c = (wq[:, qcols] * np.float32(SCALE)).astype(np.float32)
        wk_c = wk[:, c * HD + PERM]
        wv_c = wv[:, c * HD:(c + 1) * HD]
        wqkv_c = np.ascontiguousarray(
            np.concatenate([wq_c, wk_c, wv_c], axis=1)
        ).astype(np.float16)
        wo_c = np.ascontiguousarray(wo[c * QW:(c + 1) * QW, :])
        in_maps.append(
            {
                "xT": xT16,
                "wqkv": wqkv_c,
                "wo": wo_c,
                "cosT": cosT,
                "sinT": sinT,
                "tri": tri,
                "triT": triT,
                "ident": ident,
            }
        )
    return in_maps


def _assemble(results):
    out = np.zeros((L, DIM), dtype=np.float64)
    for r in results:
        out += r["out_p"].astype(np.float64)
    out = out.astype(np.float32).reshape(1, L, DIM)

    k8 = np.stack([r["k_out"][:, INVPERM] for r in results])  # [8, L, HD]
    v8 = np.stack([r["v_out"] for r in results])
    k = np.repeat(k8, NH, axis=0).reshape(1, H, L, HD).astype(np.float32)
    v = np.repeat(v8, NH, axis=0).reshape(1, H, L, HD).astype(np.float32)
    return out, k, v


def _run(inputs, trace=False):
    from concourse.bass_utils import run_bass_kernel_spmd

    nc = _build()
    in_maps = _host_prep(
        inputs["x"], inputs["wq"], inputs["wk"], inputs["wv"], inputs["wo"]
    )
    res = run_bass_kernel_spmd(
        nc, in_maps, core_ids=list(range(NCORES)), trace=trace
    )
    return _assemble(res.results), res


def kernel(x, wq, wk, wv, wo, mask):
    (out, k, v), _ = _run(
        {"x": x, "wq": wq, "wk": wk, "wv": wv, "wo": wo}
    )
    return out, k, v
